# revision 1
# baseline (speedup 1.0000x reference)
"""Int4-quantized column-parallel linear (LLaMA-7B FFN up-proj) on 8 TRN2 cores.

y[b,s,o] = sum_i x[b,s,i] * (unpack_int4(weight_q)[o,i] * scale[o]) + bias[o]

Strategy (per core, 1/8 of out_features = 1376):
  - int4 nibbles are exactly representable in fp16; matmul with integer-valued
    fp16 weights, apply scale/bias to the fp32 PSUM result at drain time.
  - x is rounded to fp16 (2^-12 relative) and the matmul accumulates in fp32
    PSUM, so the end-to-end error is ~1e-4 — far inside the 2e-2 gate — at
    full PE rate (1 cycle/row, vs 4 for native fp32 matmul).
  - weights are unpacked+transposed once into SBUF [in, feat] (moving side);
    x token-tiles are PE-transposed to [in, tok] (stationary side); PSUM out
    tile is [tok=128, feat=1376] (3 banks), drained with scale*psum+bias.
"""

from contextlib import ExitStack

import numpy as np

import concourse.bass as bass
import concourse.tile as tile
from concourse import bacc, mybir
from concourse.masks import make_identity

F32 = mybir.dt.float32
F16 = mybir.dt.float16
I32 = mybir.dt.int32

B, S, IN, OUT = 4, 2048, 4096, 11008
NCORES = 8
TOK = B * S
FEAT = OUT // NCORES

P = 128


def _feat_banks(feat):
    """Split feat into <=512 chunks (one PSUM bank each)."""
    out = []
    c0 = 0
    while c0 < feat:
        out.append((c0, min(512, feat - c0)))
        c0 += 512
    return out


def _feat_tiles(feat):
    out = []
    f0 = 0
    while f0 < feat:
        out.append((f0, min(P, feat - f0)))
        f0 += P
    return out


def build(tok=TOK, in_dim=IN, feat=FEAT):
    assert tok % P == 0 and in_dim % 256 == 0
    kp = in_dim // P       # number of 128-wide K tiles
    ntok = tok // P        # number of 128-row token tiles
    half = in_dim // 2
    banks = _feat_banks(feat)
    ftiles = _feat_tiles(feat)
    KGRP = 8                       # transposes per PSUM staging tile
    n_tg = (kp + KGRP - 1) // KGRP  # staging groups per token tile

    nc = bacc.Bacc("TRN2", target_bir_lowering=False, debug=False,
                   num_devices=NCORES)
    x_d = nc.dram_tensor("x", [tok, in_dim], F32, kind="ExternalInput").ap()
    wq_d = nc.dram_tensor("wq", [feat, half], I32, kind="ExternalInput").ap()
    sc_d = nc.dram_tensor("scale", [feat], F32, kind="ExternalInput").ap()
    bi_d = nc.dram_tensor("bias", [feat], F32, kind="ExternalInput").ap()
    y_d = nc.dram_tensor("y", [tok, feat], F32, kind="ExternalOutput").ap()

    with tile.TileContext(nc) as tc, ExitStack() as ctx:
        const = ctx.enter_context(tc.tile_pool(name="const", bufs=1))
        wtp = ctx.enter_context(tc.tile_pool(name="wt", bufs=1))
        in8k = ctx.enter_context(tc.tile_pool(name="in8k", bufs=4))
        x16p = ctx.enter_context(tc.tile_pool(name="x16", bufs=2))
        xtp = ctx.enter_context(tc.tile_pool(name="xt", bufs=2))
        outp = ctx.enter_context(tc.tile_pool(name="out", bufs=2))
        pstage = ctx.enter_context(tc.tile_pool(name="pstage", bufs=2, space="PSUM"))
        pout = ctx.enter_context(tc.tile_pool(name="pout", bufs=2, space="PSUM"))

        ident = const.tile([P, P], F16)
        make_identity(nc, ident[:])
        scale_b = const.tile([P, feat], F32)
        bias_b = const.tile([P, feat], F32)
        nc.sync.dma_start(
            out=scale_b[:],
            in_=bass.AP(tensor=sc_d.tensor, offset=sc_d.offset,
                        ap=[[0, P], sc_d.ap[0]]),
        )
        nc.sync.dma_start(
            out=bias_b[:],
            in_=bass.AP(tensor=bi_d.tensor, offset=bi_d.offset,
                        ap=[[0, P], bi_d.ap[0]]),
        )

        # Persistent dequantized+transposed weights: [in(part), k-major feat]
        wT = wtp.tile([P, kp * feat], F16)
        wTv = wT[:].rearrange("p (k f) -> p k f", k=kp)

        # ---- Phase W: unpack int4 -> fp16, transpose to [in, feat] ----
        for f0, fsz in ftiles:
            wq_t = in8k.tile([P, half], I32, tag="in8k")
            nc.sync.dma_start(out=wq_t[:fsz], in_=wq_d[f0:f0 + fsz])
            # biased nibbles: n ^ 8 maps the 2's-complement nibble to n+8
            n_lo = in8k.tile([P, half], I32, tag="in8k")
            nc.vector.tensor_scalar(
                out=n_lo[:fsz], in0=wq_t[:fsz], scalar1=15, scalar2=8,
                op0=mybir.AluOpType.bitwise_and, op1=mybir.AluOpType.bitwise_xor)
            n_hi = in8k.tile([P, half], I32, tag="in8k")
            nc.vector.tensor_scalar(
                out=n_hi[:fsz], in0=wq_t[:fsz], scalar1=4, scalar2=8,
                op0=mybir.AluOpType.logical_shift_right,
                op1=mybir.AluOpType.bitwise_xor)
            wb = in8k.tile([P, in_dim], F16, tag="in8k")
            wbv = wb[:fsz].rearrange("p (i two) -> p two i", two=2)
            # even input positions = low nibble, odd = high nibble
            nc.vector.tensor_scalar(
                out=wbv[:, 0], in0=n_lo[:fsz], scalar1=8, scalar2=None,
                op0=mybir.AluOpType.subtract)
            nc.vector.tensor_scalar(
                out=wbv[:, 1], in0=n_hi[:fsz], scalar1=8, scalar2=None,
                op0=mybir.AluOpType.subtract)
            for g in range(n_tg):
                glen = min(KGRP, kp - g * KGRP)
                st = pstage.tile([P, KGRP * P], F16)
                for j in range(glen):
                    kb = g * KGRP + j
                    nc.tensor.transpose(
                        out=st[:, j * P:j * P + fsz],
                        in_=wb[:fsz, kb * P:(kb + 1) * P],
                        identity=ident[:fsz, :fsz])
                stv = st[:].rearrange("p (j f) -> p j f", j=KGRP)
                # stage copy on ACT (reads PSUM fine) so DVE is free to run
                # the next tile's unpack in parallel
                nc.scalar.activation(
                    out=wTv[:, g * KGRP:g * KGRP + glen, f0:f0 + fsz],
                    in_=stv[:, :glen, :fsz],
                    func=mybir.ActivationFunctionType.Copy)

        # ---- Main loop: software-pipelined over token tiles ----
        # iteration i: load x(i), round to fp16, PE-transpose x(i) blocks
        # interleaved with the matmuls of token-tile i-1; drain i-1.
        state = {}

        def emit_load_round(i):
            x16 = x16p.tile([P, in_dim], F16)
            for h in range(2):
                xh = in8k.tile([P, half], F32, tag="in8k")
                nc.sync.dma_start(
                    out=xh[:], in_=x_d[i * P:(i + 1) * P, h * half:(h + 1) * half])
                hs = slice(h * half, (h + 1) * half)
                nc.scalar.activation(out=x16[:, hs], in_=xh[:],
                                     func=mybir.ActivationFunctionType.Copy)
            xt = xtp.tile([P, kp * P], F16)
            state[i] = xt
            return x16, xt

        def emit_tgroup(x16, xt, g):
            # x transposes ride the DMA xbar (2-byte dtype), on the ACT hwdge
            # queue so the SP copy queue never switches xbar mode.
            glen = min(KGRP, kp - g * KGRP)
            for j in range(glen):
                kb = g * KGRP + j
                nc.scalar.dma_start_transpose(
                    out=xt[:, kb * P:(kb + 1) * P],
                    in_=x16[:, kb * P:(kb + 1) * P])

        def emit_mm_group(i, po, ks):
            xt = state[i]
            for k in ks:
                lhsT = xt[:, k * P:(k + 1) * P]
                for c0, csz in banks:
                    nc.tensor.matmul(
                        out=po[:, c0:c0 + csz],
                        lhsT=lhsT,
                        rhs=wT[:, k * feat + c0:k * feat + c0 + csz],
                        start=(k == 0),
                        stop=(k == kp - 1))

        def emit_drain(i, po):
            ot = outp.tile([P, feat], F32)
            nc.vector.tensor_tensor(out=ot[:], in0=po[:], in1=scale_b[:],
                                    op=mybir.AluOpType.mult)
            nc.vector.tensor_tensor(out=ot[:], in0=ot[:], in1=bias_b[:],
                                    op=mybir.AluOpType.add)
            nc.sync.dma_start(out=y_d[i * P:(i + 1) * P, :], in_=ot[:])

        kchunks = np.array_split(np.arange(kp), n_tg)

        for i in range(ntok + 1):
            if i < ntok:
                x16, xt = emit_load_round(i)
            if i >= 1:
                po = pout.tile([P, feat], F32)
            for g in range(n_tg):
                if i < ntok:
                    emit_tgroup(x16, xt, g)
                if i >= 1:
                    emit_mm_group(i - 1, po, list(kchunks[g]))
            if i >= 1:
                emit_drain(i - 1, po)
                del state[i - 1]

    nc.compile()
    return nc


_CACHE = {}


def _get_program():
    if "nc" not in _CACHE:
        _CACHE["nc"] = build()
    return _CACHE["nc"]


def kernel(x, weight_q, scale, bias):
    from concourse.bass_utils import run_bass_kernel_spmd

    try:
        import jax

        jax.config.update("jax_compilation_cache_dir", "/root/problem/jax_cache")
        jax.config.update("jax_persistent_cache_min_compile_time_secs", 0)
    except Exception:
        pass

    nc = _get_program()
    xr = np.ascontiguousarray(np.asarray(x, dtype=np.float32).reshape(TOK, IN))
    wq = np.asarray(weight_q, dtype=np.int32)
    sc = np.asarray(scale, dtype=np.float32)
    bi = np.asarray(bias, dtype=np.float32)
    in_maps = []
    for c in range(NCORES):
        f0 = c * FEAT
        in_maps.append({
            "x": xr,
            "wq": np.ascontiguousarray(wq[f0:f0 + FEAT]),
            "scale": np.ascontiguousarray(sc[f0:f0 + FEAT]),
            "bias": np.ascontiguousarray(bi[f0:f0 + FEAT]),
        })
    res = run_bass_kernel_spmd(nc, in_maps, list(range(NCORES))).results
    y = np.concatenate([res[c]["y"] for c in range(NCORES)], axis=1)
    return y.reshape(B, S, OUT)



# revision 2
# speedup vs baseline: 1.4846x; 1.4846x over previous
"""Int4-quantized column-parallel linear (LLaMA-7B FFN up-proj) on 8 TRN2 cores.

y[b,s,o] = sum_i x[b,s,i] * (unpack_int4(weight_q)[o,i] * scale[o]) + bias[o]

Strategy (per core, 1/8 of out_features = 1376):
  - int4 weight nibbles are exactly representable in fp8 e4m3; x is split as
    x ~= hi + lo with hi = fp8(x16), lo = fp8(x16 - hi), so the pair of fp8
    matmuls reconstructs the fp16-rounded x exactly (end-to-end rel err ~8e-4).
  - both matmuls run in MatmulPerfMode.DoubleRow (fp8 double-pumping): each
    instruction contracts 2x128 K rows at 0.5 PE cycles per output row -> 4x
    fewer PE cycles than the fp16 path for the same GEMM.
  - x token-tiles are rounded to fp16 (ACT), DMA-transposed to [in, tok] in a
    single xbar instruction, then converted to hi/lo fp8 in transposed space,
    giving canonical contiguous [128, 2, free] DoubleRow operand APs.
  - weights are unpacked once (DVE nibble ops -> fp16), DMA-transposed to
    [in, feat], and converted to fp8 (gpsimd); scale/bias are applied at PSUM
    drain time (scale-mult on DVE, bias-add on gpsimd).
"""

from contextlib import ExitStack

import numpy as np

import concourse.bass as bass
import concourse.tile as tile
from concourse import bacc, mybir

F32 = mybir.dt.float32
F16 = mybir.dt.float16
F8 = mybir.dt.float8e4
I32 = mybir.dt.int32

B, S, IN, OUT = 4, 2048, 4096, 11008
NCORES = 8
TOK = B * S
FEAT = OUT // NCORES

P = 128
KP = IN // P          # 32 k-subtiles of 128
KSTEP = KP // 2       # 16 DoubleRow k-steps of 256
NTOK = TOK // P       # 64 token tiles
HALF = IN // 2

# PSUM output chunks: DoubleRow moving free dim <= 512 -> <=256 out columns
CHUNKS = [(c0, min(256, FEAT - c0)) for c0 in range(0, FEAT, 256)]
FTILES = [(f0, min(P, FEAT - f0)) for f0 in range(0, FEAT, P)]


def build(tok=TOK, in_dim=IN, feat=FEAT):
    assert tok % P == 0 and in_dim % 256 == 0
    ntok = tok // P
    kp = in_dim // P
    kstep = kp // 2
    half = in_dim // 2
    chunks = [(c0, min(256, feat - c0)) for c0 in range(0, feat, 256)]
    ftiles = [(f0, min(P, feat - f0)) for f0 in range(0, feat, P)]

    nc = bacc.Bacc("TRN2", target_bir_lowering=False, debug=False,
                   num_devices=NCORES)
    x_d = nc.dram_tensor("x", [tok, in_dim], F32, kind="ExternalInput").ap()
    wq_d = nc.dram_tensor("wq", [feat, half], I32, kind="ExternalInput").ap()
    sc_d = nc.dram_tensor("scale", [feat], F32, kind="ExternalInput").ap()
    bi_d = nc.dram_tensor("bias", [feat], F32, kind="ExternalInput").ap()
    y_d = nc.dram_tensor("y", [tok, feat], F32, kind="ExternalOutput").ap()

    with tile.TileContext(nc) as tc, ExitStack() as ctx:
        const = ctx.enter_context(tc.tile_pool(name="const", bufs=1))
        wtp = ctx.enter_context(tc.tile_pool(name="wt", bufs=1))
        wstg = ctx.enter_context(tc.tile_pool(name="wstg", bufs=2))
        wk8 = ctx.enter_context(tc.tile_pool(name="wk8", bufs=4))
        x32p = ctx.enter_context(tc.tile_pool(name="x32", bufs=2))
        x16p = ctx.enter_context(tc.tile_pool(name="x16", bufs=2))
        xtp = ctx.enter_context(tc.tile_pool(name="xt", bufs=2))
        hip = ctx.enter_context(tc.tile_pool(name="hi8", bufs=2))
        lop = ctx.enter_context(tc.tile_pool(name="lo8", bufs=2))
        outp = ctx.enter_context(tc.tile_pool(name="out", bufs=2))
        pout = ctx.enter_context(tc.tile_pool(name="pout", bufs=2, space="PSUM"))

        scale_b = const.tile([P, feat], F32)
        bias_b = const.tile([P, feat], F32)
        nc.sync.dma_start(
            out=scale_b[:],
            in_=bass.AP(tensor=sc_d.tensor, offset=sc_d.offset,
                        ap=[[0, P], sc_d.ap[0]]))
        nc.sync.dma_start(
            out=bias_b[:],
            in_=bass.AP(tensor=bi_d.tensor, offset=bi_d.offset,
                        ap=[[0, P], bi_d.ap[0]]))

        # Persistent fp8 weights, canonical [in(part), ksub, feat] layout
        wT8 = wtp.tile([P, kp, feat], F8)

        # ---- Phase W: unpack int4 -> fp16 -> transpose -> fp8 ----
        def emit_wtile(f0, fsz):
            wqt = wk8.tile([P, half], I32, tag="wk")
            nc.gpsimd.dma_start(out=wqt[:fsz], in_=wq_d[f0:f0 + fsz])
            n_lo = wk8.tile([P, half], I32, tag="wk")
            nc.vector.tensor_scalar(
                out=n_lo[:fsz], in0=wqt[:fsz], scalar1=15, scalar2=8,
                op0=mybir.AluOpType.bitwise_and, op1=mybir.AluOpType.bitwise_xor)
            n_hi = wk8.tile([P, half], I32, tag="wk")
            nc.vector.tensor_scalar(
                out=n_hi[:fsz], in0=wqt[:fsz], scalar1=4, scalar2=8,
                op0=mybir.AluOpType.logical_shift_right,
                op1=mybir.AluOpType.bitwise_xor)
            wb16 = wk8.tile([P, in_dim], F16, tag="wk")
            wb16v = wb16[:fsz].rearrange("p (i two) -> p two i", two=2)
            nc.vector.tensor_scalar(
                out=wb16v[:, 0], in0=n_lo[:fsz], scalar1=8, scalar2=None,
                op0=mybir.AluOpType.subtract)
            nc.vector.tensor_scalar(
                out=wb16v[:, 1], in0=n_hi[:fsz], scalar1=8, scalar2=None,
                op0=mybir.AluOpType.subtract)
            stg = wstg.tile([P, kp, P], F16)
            nc.scalar.dma_start_transpose(out=stg[:, :, :fsz], in_=wb16[:fsz])
            nc.gpsimd.tensor_copy(out=wT8[:, :, f0:f0 + fsz], in_=stg[:, :, :fsz])

        # ---- Main loop stages ----
        state = {}

        def emit_load(i):
            x32 = x32p.tile([P, in_dim], F32)
            nc.sync.dma_start(out=x32[:], in_=x_d[i * P:(i + 1) * P])
            x16 = x16p.tile([P, in_dim], F16)
            nc.scalar.activation(out=x16[:], in_=x32[:],
                                 func=mybir.ActivationFunctionType.Copy)
            xT = xtp.tile([P, kp, P], F16)
            nc.scalar.dma_start_transpose(out=xT[:], in_=x16[:])
            state[i] = (xT, None, None)

        def emit_hilo(i):
            xT, _, _ = state[i]
            hi8 = hip.tile([P, kp, P], F8)
            nc.scalar.activation(out=hi8[:], in_=xT[:],
                                 func=mybir.ActivationFunctionType.Copy)
            lo8 = lop.tile([P, kp, P], F8)
            nc.vector.tensor_tensor(out=lo8[:], in0=xT[:], in1=hi8[:],
                                    op=mybir.AluOpType.subtract)
            state[i] = (xT, hi8, lo8)

        def emit_mm(i, po):
            _, hi8, lo8 = state[i]
            for c0, csz in chunks:
                for j in range(kstep):
                    for src in (hi8, lo8):
                        nc.tensor.matmul(
                            out=po[:, c0:c0 + csz],
                            lhsT=src[:, 2 * j:2 * j + 2, :],
                            rhs=wT8[:, 2 * j:2 * j + 2, c0:c0 + csz],
                            start=(j == 0 and src is hi8),
                            stop=(j == kstep - 1 and src is lo8),
                            perf_mode=mybir.MatmulPerfMode.DoubleRow)

        def emit_drain(i, po):
            ot = outp.tile([P, feat], F32)
            nc.vector.tensor_tensor(out=ot[:], in0=po[:], in1=scale_b[:],
                                    op=mybir.AluOpType.mult)
            nc.gpsimd.tensor_tensor(out=ot[:], in0=ot[:], in1=bias_b[:],
                                    op=mybir.AluOpType.add)
            nc.gpsimd.dma_start(out=y_d[i * P:(i + 1) * P, :], in_=ot[:])

        for f0, fsz in ftiles:
            emit_wtile(f0, fsz)

        for i in range(ntok + 2):
            if i < ntok:
                emit_load(i)
            if 1 <= i <= ntok:
                emit_hilo(i - 1)
            if i >= 2:
                po = pout.tile([P, feat], F32)
                emit_mm(i - 2, po)
                emit_drain(i - 2, po)
                del state[i - 2]

    nc.compile()
    return nc


_CACHE = {}


def _get_program():
    if "nc" not in _CACHE:
        _CACHE["nc"] = build()
    return _CACHE["nc"]


def kernel(x, weight_q, scale, bias):
    from concourse.bass_utils import run_bass_kernel_spmd

    try:
        import jax

        jax.config.update("jax_compilation_cache_dir", "/root/problem/jax_cache")
        jax.config.update("jax_persistent_cache_min_compile_time_secs", 0)
    except Exception:
        pass

    nc = _get_program()
    xr = np.ascontiguousarray(np.asarray(x, dtype=np.float32).reshape(TOK, IN))
    wq = np.asarray(weight_q, dtype=np.int32)
    sc = np.asarray(scale, dtype=np.float32)
    bi = np.asarray(bias, dtype=np.float32)
    in_maps = []
    for c in range(NCORES):
        f0 = c * FEAT
        in_maps.append({
            "x": xr,
            "wq": np.ascontiguousarray(wq[f0:f0 + FEAT]),
            "scale": np.ascontiguousarray(sc[f0:f0 + FEAT]),
            "bias": np.ascontiguousarray(bi[f0:f0 + FEAT]),
        })
    res = run_bass_kernel_spmd(nc, in_maps, list(range(NCORES))).results
    y = np.concatenate([res[c]["y"] for c in range(NCORES)], axis=1)
    return y.reshape(B, S, OUT)
